# revision 24
# baseline (speedup 1.0000x reference)
"""DualMem retrieval kernel for Trainium2 (8 NeuronCores, Bass/Tile).

Math (per reference):
    sim[b,c,m]  = <img[b], mem[c,m]>
    w           = exp(-beta * (1 - sim))
    adapt[b,c]  = sum_m mem[c,m] * w[b,c,m]
    logits[b,c] = 100 * <img[b], adapt[b,c] / ||adapt[b,c]||>

Key algebraic reduction (avoids materializing adapt [B,C,D]):
    numer[b,c]  = <img[b], adapt[b,c]> = sum_m w[b,c,m] * sim[b,c,m]
    denom[b,c]  = ||adapt[b,c]||^2     = w^T G_c w,  G_c = mem_c @ mem_c^T  (11x11 Gram)
    logits      = 100 * numer / sqrt(denom)

Sharding: classes C=1000 split 125 per core across 8 cores (mem bank fully
sharded; only img replicated).

Per-core layout: groups of 11 classes x 11 memories = 121 partitions (pad to
128); 12 groups cover 132 >= 125 class slots.  The bf16 memory bank arrives
class-major and is xbar-DMA-transposed on-chip to [d, cm].  Groups are
processed in blocks of 4 sharing PSUM banks (per-element has_written makes
disjoint column ranges in one bank legal accumulation groups; the bank-level
software check is skipped):
    su bank  [128, 512]: per group k, cols 128k+0:64  = sim (acc over d)
                                      cols 128k+64:128 = u = G_masked^T w
    G bank   [128, 512]: per group k, cols 128k : 128k+128 = Gram (acc over d)
Downstream per block: one batched exp, one masked-Gram copy, four u-matmuls,
two strided muls building [w*sim | w*u], one 0/1 "E" matmul summing over m
per class -> [numer | denom], then one batched sqrt/recip/mul at the end.
"""

import os
import sys

sys.path.insert(0, "/opt/trn_rl_repo")

import ml_dtypes
import numpy as np

B, C, M, D = 64, 1000, 11, 1024
BETA = 5.5
N_CORES = 8
C_PER = C // N_CORES          # 125 classes per core
CPG = 11                      # classes per group
NG = 12                       # groups per core (132 class slots >= 125)
PG = CPG * M                  # 121 used partitions per group
DCH = D // 128                # 8 d-chunks
ROWS = NG * 128               # 1536 class-major rows per core
GPB = 4                       # groups per PSUM block
NB = NG // GPB                # 3 blocks

_cache = {}


def _build():
    import concourse.mybir as mybir
    import concourse.tile as tile
    from concourse import bacc

    f32 = mybir.dt.float32
    bf16 = mybir.dt.bfloat16

    nc = bacc.Bacc("TRN2", target_bir_lowering=False, debug=False,
                   num_devices=N_CORES)

    # membf rows: [64 img rows | 1536 class-major memory rows]; the xbar
    # transpose of the first 64 rows lands imgT in exactly the layout the
    # sim matmuls want, so img needs no separate load.
    membf = nc.dram_tensor("membf", [64 + ROWS, D], bf16, kind="ExternalInput")
    mask = nc.dram_tensor("mask", [128, GPB * 128], f32, kind="ExternalInput")
    em = nc.dram_tensor("em", [128, 16], f32, kind="ExternalInput")
    out = nc.dram_tensor("out", [16, NG * 64], f32, kind="ExternalOutput")

    with tile.TileContext(nc) as tc:
        with (
            tc.tile_pool(name="const", bufs=1) as const,
            tc.tile_pool(name="sb", bufs=2) as sb,
            tc.tile_pool(name="ps_su", bufs=2, space="PSUM") as ps_su,
            tc.tile_pool(name="ps_g", bufs=2, space="PSUM") as ps_g,
            tc.tile_pool(name="ps_nd", bufs=2, space="PSUM") as ps_nd,
        ):
            # memT[d % 128, d_chunk, cm]; one tile per block of 4 groups.
            # Tile 0 carries 64 extra leading cm-columns holding imgT.
            mt = [const.tile([128, DCH, (64 if q == 0 else 0) + GPB * 128],
                             bf16, name=f"mt{q}", tag=f"mt{q}")
                  for q in range(NB)]
            it = mt[0][:, :, 0:64]
            mask_sb = const.tile([128, GPB * 128], f32)
            em_sb = const.tile([128, 16], f32)
            ndall = const.tile([16, NG * 128], f32)
            lg = const.tile([16, NG * 64], f32)
            bias_exp = const.tile([128, 1], f32)
            bias_eps = const.tile([16, 1], f32)
            nc.vector.memset(bias_exp[:], -BETA)
            nc.vector.memset(bias_eps[:], 1e-30)

            # xbar transposes first so compute starts ASAP; mask/em (needed
            # only mid-pipeline) follow.  Keeping all plain copies after all
            # transposes avoids repeated xbar-mode serialization stalls.
            nc.gpsimd.dma_start(mask_sb[:], mask.ap())
            nc.gpsimd.dma_start(em_sb[:], em.ap())
            H = ROWS // NB
            for q in range(NB):
                ext = 64 if q == 0 else 0
                r0 = 0 if q == 0 else 64 + q * H
                nc.sync.dma_start(
                    mt[q][:],
                    membf.ap()[r0:64 + (q + 1) * H, :],
                    transpose=True,
                )

            for nb in range(NB):
                su = ps_su.tile([128, GPB * 128], f32)
                gp = ps_g.tile([128, GPB * 128], f32)
                ext = 64 if nb == 0 else 0
                for k in range(GPB):
                    for i in range(DCH):
                        blk = mt[nb][:, i, ext + k * 128:ext + (k + 1) * 128]
                        nc.tensor.matmul(su[:, k * 128:k * 128 + 64],
                                         blk, it[:, i, :],
                                         start=(i == 0), stop=(i == DCH - 1),
                                         skip_group_check=True)
                        nc.tensor.matmul(gp[:, k * 128:(k + 1) * 128],
                                         blk, blk,
                                         start=(i == 0), stop=(i == DCH - 1),
                                         skip_group_check=True)

                # w = exp(beta*sim - beta) for all 4 groups at once
                su4 = su[:].rearrange("p (k t b) -> p k t b", k=GPB, t=2)
                w4 = sb.tile([128, GPB * 64], f32, tag="w4")
                nc.scalar.activation(w4[:], su4[:, :, 0, :],
                                     mybir.ActivationFunctionType.Exp,
                                     bias=bias_exp[:], scale=BETA)

                # masked Gram -> SBUF (kills cross-class + pad entries)
                gm4 = sb.tile([128, GPB * 128], f32, tag="gm4")
                nc.vector.tensor_mul(gm4[:], gp[:], mask_sb[:])

                # u_k = G_k^T @ w_k, placed next to sim_k in the same bank
                for k in range(GPB):
                    nc.tensor.matmul(su[:, k * 128 + 64:(k + 1) * 128],
                                     gm4[:, k * 128:(k + 1) * 128],
                                     w4[:, k * 64:(k + 1) * 64],
                                     start=True, stop=True,
                                     skip_group_check=True)

                # wsq = [w*sim | w*u] in group-blocked layout (two strided muls)
                wsq = sb.tile([128, GPB * 128], f32, tag="wsq")
                wq4 = wsq[:].rearrange("p (k t b) -> p k t b", k=GPB, t=2)
                nc.vector.tensor_mul(wq4[:, :, 0, :], su4[:, :, 0, :], w4[:])
                nc.vector.tensor_mul(wq4[:, :, 1, :], su4[:, :, 1, :], w4[:])

                # nd[c, :] = [numer | denom] per class for the whole block
                nd = ps_nd.tile([16, GPB * 128], f32)
                nc.tensor.matmul(nd[:], em_sb[:], wsq[:],
                                 start=True, stop=True)
                nc.vector.tensor_copy(
                    ndall[:, nb * GPB * 128:(nb + 1) * GPB * 128], nd[:])

            # logits = numer * 100/sqrt(denom), batched over all groups
            nd3 = ndall[:].rearrange("p (g t b) -> p g t b", g=NG, t=2)
            s_all = sb.tile([16, NG * 64], f32, tag="s")
            nc.scalar.activation(s_all[:], nd3[:, :, 1, :],
                                 mybir.ActivationFunctionType.Sqrt,
                                 bias=bias_eps[:], scale=1e-4)
            r_all = sb.tile([16, NG * 64], f32, tag="r")
            nc.vector.reciprocal(r_all[:], s_all[:])
            nc.vector.tensor_mul(lg[:], nd3[:, :, 0, :], r_all[:])

            nc.sync.dma_start(out.ap(), lg[:])

    nc.compile()
    return nc


def _get_nc():
    if "nc" not in _cache:
        _cache["nc"] = _build()
    return _cache["nc"]


def _prep_inputs(img_features, memorized_image_feat):
    """Host-side formatting: bf16 cast, class padding, group layout."""
    bf = ml_dtypes.bfloat16
    img_b = np.ascontiguousarray(img_features.astype(bf))          # [64, 1024]
    mem_b = memorized_image_feat.astype(bf)                        # [1000,11,1024]

    m1 = np.zeros((128, 128), np.float32)
    for c in range(CPG):
        m1[c * M:(c + 1) * M, c * M:(c + 1) * M] = 1.0
    mask = np.zeros((128, GPB * 128), np.float32)
    for k in range(GPB):
        mask[:, k * 128:(k + 1) * 128] = m1
    em = np.zeros((128, 16), np.float32)
    for c in range(CPG):
        em[c * M:(c + 1) * M, c] = 1.0

    in_maps = []
    for k in range(N_CORES):
        sl = mem_b[k * C_PER:(k + 1) * C_PER]                      # [125,11,1024]
        pad = np.zeros((NG * CPG, M, D), bf)
        pad[:C_PER] = sl
        grp = pad.reshape(NG, PG, D)
        full = np.zeros((NG, 128, D), bf)
        full[:, :PG] = grp
        membf = np.empty((64 + ROWS, D), bf)
        membf[:64] = img_b
        membf[64:] = full.reshape(ROWS, D)
        in_maps.append({"membf": membf, "mask": mask, "em": em})
    return in_maps


def _gather(results):
    logits = np.empty((B, C), np.float32)
    for k in range(N_CORES):
        o = results[k]["out"].reshape(16, NG, 64)[:CPG]            # [11, 12, 64]
        o = o.transpose(1, 0, 2).reshape(NG * CPG, 64)[:C_PER]     # [125, 64]
        logits[:, k * C_PER:(k + 1) * C_PER] = o.T
    return logits


def kernel(img_features, memorized_image_feat):
    from concourse.bass_utils import run_bass_kernel_spmd

    nc = _get_nc()
    in_maps = _prep_inputs(img_features, memorized_image_feat)
    res = run_bass_kernel_spmd(nc, in_maps, core_ids=list(range(N_CORES)))
    return _gather(res.results)


# revision 29
# speedup vs baseline: 1.1846x; 1.1846x over previous
"""DualMem retrieval kernel for Trainium2 (8 NeuronCores, Bass/Tile).

Math (per reference):
    sim[b,c,m]  = <img[b], mem[c,m]>
    w           = exp(-beta * (1 - sim))
    adapt[b,c]  = sum_m mem[c,m] * w[b,c,m]
    logits[b,c] = 100 * <img[b], adapt[b,c] / ||adapt[b,c]||>

Key algebraic reduction (avoids materializing adapt [B,C,D]):
    numer[b,c]  = <img[b], adapt[b,c]> = sum_m w[b,c,m] * sim[b,c,m]
    denom[b,c]  = ||adapt[b,c]||^2     = w^T G_c w,  G_c = mem_c @ mem_c^T  (11x11 Gram)
    logits      = 100 * numer / sqrt(denom)

Sharding: classes C=1000 split 125 per core across 8 cores (mem bank fully
sharded; only img replicated).

Per-core layout: groups of 11 classes x 11 memories = 121 partitions (pad to
128); 12 groups cover 132 >= 125 class slots.  The bf16 memory bank arrives
class-major and is xbar-DMA-transposed on-chip to [d, cm].  Groups are
processed in blocks of 4 sharing PSUM banks (per-element has_written makes
disjoint column ranges in one bank legal accumulation groups; the bank-level
software check is skipped):
    su bank  [128, 512]: per group k, cols 128k+0:64  = sim (acc over d)
                                      cols 128k+64:128 = u = G_masked^T w
    G bank   [128, 512]: per group k, cols 128k : 128k+128 = Gram (acc over d)
Downstream per block: one batched exp, one masked-Gram copy, four u-matmuls,
two strided muls building [w*sim | w*u], one 0/1 "E" matmul summing over m
per class -> [numer | denom], then one batched sqrt/recip/mul at the end.
"""

import os
import sys

sys.path.insert(0, "/opt/trn_rl_repo")

import ml_dtypes
import numpy as np

B, C, M, D = 64, 1000, 11, 1024
BETA = 5.5
N_CORES = 8
C_PER = C // N_CORES          # 125 classes per core
CPG = 11                      # classes per group
NG = 12                       # groups per core (132 class slots >= 125)
PG = CPG * M                  # 121 used partitions per group
DCH = D // 128                # 8 d-chunks
ROWS = NG * 128               # 1536 class-major rows per core
GPB = 4                       # groups per PSUM block
NB = NG // GPB                # 3 blocks

_cache = {}


def _build():
    import concourse.mybir as mybir
    import concourse.tile as tile
    from concourse import bacc

    f32 = mybir.dt.float32
    bf16 = mybir.dt.bfloat16

    nc = bacc.Bacc("TRN2", target_bir_lowering=False, debug=False,
                   num_devices=N_CORES)

    # membf rows: [64 img | 128 mask/em | 1536 class-major memory rows].
    # The xbar transpose of the leading rows lands imgT, the block-diagonal
    # Gram mask, and the 0/1 class-sum matrix (all bf16-exact) in exactly
    # the layouts the compute wants — no separate const loads at all.
    EXT = 192
    membf = nc.dram_tensor("membf", [EXT + ROWS, D], bf16, kind="ExternalInput")
    out = nc.dram_tensor("out", [16, NG * 64], f32, kind="ExternalOutput")

    with tile.TileContext(nc) as tc:
        with (
            tc.tile_pool(name="const", bufs=1) as const,
            tc.tile_pool(name="sb", bufs=2) as sb,
            tc.tile_pool(name="ps_su", bufs=2, space="PSUM") as ps_su,
            tc.tile_pool(name="ps_g", bufs=2, space="PSUM") as ps_g,
            tc.tile_pool(name="ps_nd", bufs=2, space="PSUM") as ps_nd,
        ):
            # memT[d % 128, d_chunk, cm]; one tile per block of 4 groups.
            # Tile 0 carries EXT extra leading cm-columns: 0:64 imgT, then
            # the transposed mask/em rows (mask in d-chunks 0-3, em chunk 4).
            mt = [const.tile([128, DCH, (EXT if q == 0 else 0) + GPB * 128],
                             bf16, name=f"mt{q}", tag=f"mt{q}")
                  for q in range(NB)]
            it = mt[0][:, :, 0:64]
            mask_bf = mt[0][:, 0:4, 64:192]          # [128, 4, 128] bf16
            em_bf = mt[0][:, 4, 64:80]               # [128, 16] bf16
            mask_sb = const.tile([128, GPB * 128], f32)
            em_sb = const.tile([128, 16], f32)
            ndall = const.tile([16, NG * 128], f32)
            lg = const.tile([16, NG * 64], f32)
            bias_exp = const.tile([128, 1], f32)
            bias_eps = const.tile([16, 1], f32)
            nc.vector.memset(bias_exp[:], -BETA)
            nc.vector.memset(bias_eps[:], 1e-30)

            # xbar transposes first so compute starts ASAP; mask/em (needed
            # only mid-pipeline) follow.  Keeping all plain copies after all
            # transposes avoids repeated xbar-mode serialization stalls.
            H = ROWS // NB
            for q in range(NB):
                r0 = 0 if q == 0 else EXT + q * H
                nc.sync.dma_start(
                    mt[q][:],
                    membf.ap()[r0:EXT + (q + 1) * H, :],
                    transpose=True,
                )
            # one-time f32 casts of the streamed consts
            nc.vector.tensor_copy(mask_sb[:], mask_bf)
            nc.vector.tensor_copy(em_sb[:], em_bf)

            for nb in range(NB):
                su = ps_su.tile([128, GPB * 128], f32)
                gp = ps_g.tile([128, GPB * 128], f32)
                ext = EXT if nb == 0 else 0
                for k in range(GPB):
                    for i in range(DCH):
                        blk = mt[nb][:, i, ext + k * 128:ext + (k + 1) * 128]
                        nc.tensor.matmul(su[:, k * 128:k * 128 + 64],
                                         blk, it[:, i, :],
                                         start=(i == 0), stop=(i == DCH - 1),
                                         skip_group_check=True)
                        nc.tensor.matmul(gp[:, k * 128:(k + 1) * 128],
                                         blk, blk,
                                         start=(i == 0), stop=(i == DCH - 1),
                                         skip_group_check=True)

                # w = exp(beta*sim - beta) for all 4 groups at once
                su4 = su[:].rearrange("p (k t b) -> p k t b", k=GPB, t=2)
                w4 = sb.tile([128, GPB * 64], f32, tag="w4")
                nc.scalar.activation(w4[:], su4[:, :, 0, :],
                                     mybir.ActivationFunctionType.Exp,
                                     bias=bias_exp[:], scale=BETA)

                # masked Gram -> SBUF (kills cross-class + pad entries)
                gm4 = sb.tile([128, GPB * 128], f32, tag="gm4")
                nc.vector.tensor_mul(gm4[:], gp[:], mask_sb[:])

                # u_k = G_k^T @ w_k, placed next to sim_k in the same bank
                for k in range(GPB):
                    nc.tensor.matmul(su[:, k * 128 + 64:(k + 1) * 128],
                                     gm4[:, k * 128:(k + 1) * 128],
                                     w4[:, k * 64:(k + 1) * 64],
                                     start=True, stop=True,
                                     skip_group_check=True)

                # wsq = [w*sim | w*u] in group-blocked layout (two strided muls)
                wsq = sb.tile([128, GPB * 128], f32, tag="wsq")
                wq4 = wsq[:].rearrange("p (k t b) -> p k t b", k=GPB, t=2)
                nc.vector.tensor_mul(wq4[:, :, 0, :], su4[:, :, 0, :], w4[:])
                nc.vector.tensor_mul(wq4[:, :, 1, :], su4[:, :, 1, :], w4[:])

                # nd[c, :] = [numer | denom] per class for the whole block
                nd = ps_nd.tile([16, GPB * 128], f32)
                nc.tensor.matmul(nd[:], em_sb[:], wsq[:],
                                 start=True, stop=True)
                nc.vector.tensor_copy(
                    ndall[:, nb * GPB * 128:(nb + 1) * GPB * 128], nd[:])

            # logits = numer * 100/sqrt(denom), batched over all groups
            nd3 = ndall[:].rearrange("p (g t b) -> p g t b", g=NG, t=2)
            s_all = sb.tile([16, NG * 64], f32, tag="s")
            nc.scalar.activation(s_all[:], nd3[:, :, 1, :],
                                 mybir.ActivationFunctionType.Sqrt,
                                 bias=bias_eps[:], scale=1e-4)
            r_all = sb.tile([16, NG * 64], f32, tag="r")
            nc.vector.reciprocal(r_all[:], s_all[:])
            nc.vector.tensor_mul(lg[:], nd3[:, :, 0, :], r_all[:])

            nc.sync.dma_start(out.ap(), lg[:])

    nc.compile()
    return nc


def _get_nc():
    if "nc" not in _cache:
        _cache["nc"] = _build()
    return _cache["nc"]


def _prep_inputs(img_features, memorized_image_feat):
    """Host-side formatting: bf16 cast, class padding, group layout."""
    bf = ml_dtypes.bfloat16
    img_b = np.ascontiguousarray(img_features.astype(bf))          # [64, 1024]
    mem_b = memorized_image_feat.astype(bf)                        # [1000,11,1024]

    m1 = np.zeros((128, 128), np.float32)
    for c in range(CPG):
        m1[c * M:(c + 1) * M, c * M:(c + 1) * M] = 1.0
    em = np.zeros((128, 16), np.float32)
    for c in range(CPG):
        em[c * M:(c + 1) * M, c] = 1.0

    # mask/em rows for the transpose stream: transposing maskem[j, 128i+p]
    # yields m1 at d-chunks 0-3 and em^T at chunk 4
    maskem = np.zeros((128, D), bf)
    for i in range(4):
        maskem[:, i * 128:(i + 1) * 128] = m1.T
    maskem[:16, 512:640] = em.T

    in_maps = []
    for k in range(N_CORES):
        sl = mem_b[k * C_PER:(k + 1) * C_PER]                      # [125,11,1024]
        pad = np.zeros((NG * CPG, M, D), bf)
        pad[:C_PER] = sl
        grp = pad.reshape(NG, PG, D)
        full = np.zeros((NG, 128, D), bf)
        full[:, :PG] = grp
        membf = np.empty((192 + ROWS, D), bf)
        membf[:64] = img_b
        membf[64:192] = maskem
        membf[192:] = full.reshape(ROWS, D)
        in_maps.append({"membf": membf})
    return in_maps


def _gather(results):
    logits = np.empty((B, C), np.float32)
    for k in range(N_CORES):
        o = results[k]["out"].reshape(16, NG, 64)[:CPG]            # [11, 12, 64]
        o = o.transpose(1, 0, 2).reshape(NG * CPG, 64)[:C_PER]     # [125, 64]
        logits[:, k * C_PER:(k + 1) * C_PER] = o.T
    return logits


def kernel(img_features, memorized_image_feat):
    from concourse.bass_utils import run_bass_kernel_spmd

    nc = _get_nc()
    in_maps = _prep_inputs(img_features, memorized_image_feat)
    res = run_bass_kernel_spmd(nc, in_maps, core_ids=list(range(N_CORES)))
    return _gather(res.results)


# revision 34
# speedup vs baseline: 1.2739x; 1.0754x over previous
"""DualMem retrieval kernel for Trainium2 (8 NeuronCores, Bass/Tile).

Math (per reference):
    sim[b,c,m]  = <img[b], mem[c,m]>
    w           = exp(-beta * (1 - sim))
    adapt[b,c]  = sum_m mem[c,m] * w[b,c,m]
    logits[b,c] = 100 * <img[b], adapt[b,c] / ||adapt[b,c]||>

Key algebraic reduction (avoids materializing adapt [B,C,D]):
    numer[b,c]  = <img[b], adapt[b,c]> = sum_m w[b,c,m] * sim[b,c,m]
    denom[b,c]  = ||adapt[b,c]||^2     = w^T G_c w,  G_c = mem_c @ mem_c^T  (11x11 Gram)
    logits      = 100 * numer / sqrt(denom)

Sharding: classes C=1000 split 125 per core across 8 cores (mem bank fully
sharded; only img replicated).

Per-core layout: groups of 11 classes x 11 memories = 121 partitions (pad to
128); 12 groups cover 132 >= 125 class slots.  The bf16 memory bank arrives
class-major and is xbar-DMA-transposed on-chip to [d, cm].  Groups are
processed in blocks of 4 sharing PSUM banks (per-element has_written makes
disjoint column ranges in one bank legal accumulation groups; the bank-level
software check is skipped):
    su bank  [128, 512]: per group k, cols 128k+0:64  = sim (acc over d)
                                      cols 128k+64:128 = u = G_masked^T w
    G bank   [128, 512]: per group k, cols 128k : 128k+128 = Gram (acc over d)
Downstream per block: one batched exp, one masked-Gram copy, four u-matmuls,
two strided muls building [w*sim | w*u], one 0/1 "E" matmul summing over m
per class -> [numer | denom], then one batched sqrt/recip/mul at the end.
"""

import os
import sys

sys.path.insert(0, "/opt/trn_rl_repo")

import ml_dtypes
import numpy as np

B, C, M, D = 64, 1000, 11, 1024
BETA = 5.5
N_CORES = 8
C_PER = C // N_CORES          # 125 classes per core
CPG = 11                      # classes per group
NG = 12                       # groups per core (132 class slots >= 125)
PG = CPG * M                  # 121 used partitions per group
DCH = D // 128                # 8 d-chunks
ROWS = NG * 128               # 1536 class-major rows per core
GPB = 4                       # groups per PSUM block
NB = NG // GPB                # 3 blocks

_cache = {}


def _build():
    import concourse.mybir as mybir
    import concourse.tile as tile
    from concourse import bacc

    f32 = mybir.dt.float32
    bf16 = mybir.dt.bfloat16

    nc = bacc.Bacc("TRN2", target_bir_lowering=False, debug=False,
                   num_devices=N_CORES)

    # membf rows: [64 img | 128 mask/em | 1536 class-major memory rows].
    # The xbar transpose of the leading rows lands imgT, the block-diagonal
    # Gram mask, and the 0/1 class-sum matrix (all bf16-exact) in exactly
    # the layouts the compute wants — no separate const loads at all.
    EXT = 192
    membf = nc.dram_tensor("membf", [EXT + ROWS, D], bf16, kind="ExternalInput")
    out = nc.dram_tensor("out", [16, NG * 64], f32, kind="ExternalOutput")

    with tile.TileContext(nc) as tc:
        with (
            tc.tile_pool(name="const", bufs=1) as const,
            tc.tile_pool(name="sb", bufs=2) as sb,
            tc.tile_pool(name="ps_su", bufs=2, space="PSUM") as ps_su,
            tc.tile_pool(name="ps_g", bufs=2, space="PSUM") as ps_g,
            tc.tile_pool(name="ps_nd", bufs=1, space="PSUM") as ps_nd,
        ):
            # memT[d % 128, d_chunk, cm]; one tile per transpose batch of 2
            # groups.  Tile 0 carries EXT extra leading cm-columns: 0:64
            # imgT, then transposed mask/em rows (mask d-chunks 0-3, em 4).
            NT = NG // 2
            mt = [const.tile([128, DCH, (EXT if q == 0 else 0) + 256],
                             bf16, name=f"mt{q}", tag=f"mt{q}")
                  for q in range(NT)]
            it = mt[0][:, :, 0:64]
            mask_bf = mt[0][:, 0:4, 64:192]          # [128, 4, 128] bf16
            em_bf = mt[0][:, 4, 64:80]               # [128, 16] bf16
            mask_sb = const.tile([128, GPB * 128], f32)
            em_sb = const.tile([128, 16], f32)
            lg = const.tile([16, NG * 64], f32)
            bias_exp = const.tile([128, 1], f32)
            bias_eps = const.tile([16, 1], f32)
            nc.vector.memset(bias_exp[:], -BETA)
            nc.vector.memset(bias_eps[:], 1e-30)

            # xbar transposes first so compute starts ASAP; mask/em (needed
            # only mid-pipeline) follow.  Keeping all plain copies after all
            # transposes avoids repeated xbar-mode serialization stalls.
            for q in range(NT):
                r0 = 0 if q == 0 else EXT + q * 256
                nc.sync.dma_start(
                    mt[q][:],
                    membf.ap()[r0:EXT + (q + 1) * 256, :],
                    transpose=True,
                )
            # one-time f32 casts of the streamed consts
            nc.vector.tensor_copy(mask_sb[:], mask_bf)
            nc.vector.tensor_copy(em_sb[:], em_bf)

            # [numer | denom] for all 12 groups: one 3-bank PSUM tile; each
            # E-matmul writes exactly one bank-aligned [16, 512] slice
            nd_all = ps_nd.tile([16, NG * 128], f32)

            for nb in range(NB):
                su = ps_su.tile([128, GPB * 128], f32)
                gp = ps_g.tile([128, GPB * 128], f32)
                for k in range(GPB):
                    g = nb * GPB + k
                    q, r = divmod(g, 2)
                    ext = EXT if q == 0 else 0
                    for i in range(DCH):
                        blk = mt[q][:, i, ext + r * 128:ext + (r + 1) * 128]
                        nc.tensor.matmul(su[:, k * 128:k * 128 + 64],
                                         blk, it[:, i, :],
                                         start=(i == 0), stop=(i == DCH - 1),
                                         skip_group_check=True)
                        nc.tensor.matmul(gp[:, k * 128:(k + 1) * 128],
                                         blk, blk,
                                         start=(i == 0), stop=(i == DCH - 1),
                                         skip_group_check=True)

                # w = exp(beta*sim - beta) for all 4 groups at once
                su4 = su[:].rearrange("p (k t b) -> p k t b", k=GPB, t=2)
                w4 = sb.tile([128, GPB * 64], f32, tag="w4")
                nc.scalar.activation(w4[:], su4[:, :, 0, :],
                                     mybir.ActivationFunctionType.Exp,
                                     bias=bias_exp[:], scale=BETA)

                # masked Gram -> SBUF (kills cross-class + pad entries)
                gm4 = sb.tile([128, GPB * 128], f32, tag="gm4")
                nc.vector.tensor_mul(gm4[:], gp[:], mask_sb[:])

                # u_k = G_k^T @ w_k, placed next to sim_k in the same bank
                for k in range(GPB):
                    nc.tensor.matmul(su[:, k * 128 + 64:(k + 1) * 128],
                                     gm4[:, k * 128:(k + 1) * 128],
                                     w4[:, k * 64:(k + 1) * 64],
                                     start=True, stop=True,
                                     skip_group_check=True)

                # wsq = [w*sim | w*u] in group-blocked layout (two strided muls)
                wsq = sb.tile([128, GPB * 128], f32, tag="wsq")
                wq4 = wsq[:].rearrange("p (k t b) -> p k t b", k=GPB, t=2)
                nc.vector.tensor_mul(wq4[:, :, 0, :], su4[:, :, 0, :], w4[:])
                nc.vector.tensor_mul(wq4[:, :, 1, :], su4[:, :, 1, :], w4[:])

                # nd[c, :] = [numer | denom] per class for the whole block
                nc.tensor.matmul(
                    nd_all[:, nb * GPB * 128:(nb + 1) * GPB * 128],
                    em_sb[:], wsq[:], start=True, stop=True,
                    skip_group_check=True)

            # logits = numer * 100/sqrt(denom), batched over all groups,
            # reading [numer | denom] straight out of PSUM
            nd3 = nd_all[:].rearrange("p (g t b) -> p g t b", g=NG, t=2)
            s_all = sb.tile([16, NG * 64], f32, tag="s")
            nc.scalar.activation(s_all[:], nd3[:, :, 1, :],
                                 mybir.ActivationFunctionType.Sqrt,
                                 bias=bias_eps[:], scale=1e-4)
            r_all = sb.tile([16, NG * 64], f32, tag="r")
            nc.vector.reciprocal(r_all[:], s_all[:])
            nc.vector.tensor_mul(lg[:], nd3[:, :, 0, :], r_all[:])

            nc.sync.dma_start(out.ap(), lg[:])

    nc.compile()
    return nc


def _get_nc():
    if "nc" not in _cache:
        _cache["nc"] = _build()
    return _cache["nc"]


def _prep_inputs(img_features, memorized_image_feat):
    """Host-side formatting: bf16 cast, class padding, group layout."""
    bf = ml_dtypes.bfloat16
    img_b = np.ascontiguousarray(img_features.astype(bf))          # [64, 1024]
    mem_b = memorized_image_feat.astype(bf)                        # [1000,11,1024]

    m1 = np.zeros((128, 128), np.float32)
    for c in range(CPG):
        m1[c * M:(c + 1) * M, c * M:(c + 1) * M] = 1.0
    em = np.zeros((128, 16), np.float32)
    for c in range(CPG):
        em[c * M:(c + 1) * M, c] = 1.0

    # mask/em rows for the transpose stream: transposing maskem[j, 128i+p]
    # yields m1 at d-chunks 0-3 and em^T at chunk 4
    maskem = np.zeros((128, D), bf)
    for i in range(4):
        maskem[:, i * 128:(i + 1) * 128] = m1.T
    maskem[:16, 512:640] = em.T

    in_maps = []
    for k in range(N_CORES):
        sl = mem_b[k * C_PER:(k + 1) * C_PER]                      # [125,11,1024]
        pad = np.zeros((NG * CPG, M, D), bf)
        pad[:C_PER] = sl
        grp = pad.reshape(NG, PG, D)
        full = np.zeros((NG, 128, D), bf)
        full[:, :PG] = grp
        membf = np.empty((192 + ROWS, D), bf)
        membf[:64] = img_b
        membf[64:192] = maskem
        membf[192:] = full.reshape(ROWS, D)
        in_maps.append({"membf": membf})
    return in_maps


def _gather(results):
    logits = np.empty((B, C), np.float32)
    for k in range(N_CORES):
        o = results[k]["out"].reshape(16, NG, 64)[:CPG]            # [11, 12, 64]
        o = o.transpose(1, 0, 2).reshape(NG * CPG, 64)[:C_PER]     # [125, 64]
        logits[:, k * C_PER:(k + 1) * C_PER] = o.T
    return logits


def kernel(img_features, memorized_image_feat):
    from concourse.bass_utils import run_bass_kernel_spmd

    nc = _get_nc()
    in_maps = _prep_inputs(img_features, memorized_image_feat)
    res = run_bass_kernel_spmd(nc, in_maps, core_ids=list(range(N_CORES)))
    return _gather(res.results)


# revision 39
# speedup vs baseline: 1.4039x; 1.1020x over previous
"""DualMem retrieval kernel for Trainium2 (8 NeuronCores, Bass/Tile).

Math (per reference):
    sim[b,c,m]  = <img[b], mem[c,m]>
    w           = exp(-beta * (1 - sim))
    adapt[b,c]  = sum_m mem[c,m] * w[b,c,m]
    logits[b,c] = 100 * <img[b], adapt[b,c] / ||adapt[b,c]||>

Key algebraic reduction (avoids materializing adapt [B,C,D]):
    numer[b,c]  = <img[b], adapt[b,c]> = sum_m w[b,c,m] * sim[b,c,m]
    denom[b,c]  = ||adapt[b,c]||^2     = w^T G_c w,  G_c = mem_c @ mem_c^T  (11x11 Gram)
    logits      = 100 * numer / sqrt(denom)

Sharding: classes C=1000 split 125 per core across 8 cores (mem bank fully
sharded; only img replicated).

Per-core layout: groups of 11 classes x 11 memories = 121 partitions (pad to
128); 12 groups cover 132 >= 125 class slots.  The bf16 memory bank arrives
class-major and is xbar-DMA-transposed on-chip to [d, cm].  Groups are
processed in blocks of 4 sharing PSUM banks (per-element has_written makes
disjoint column ranges in one bank legal accumulation groups; the bank-level
software check is skipped):
    su bank  [128, 512]: per group k, cols 128k+0:64  = sim (acc over d)
                                      cols 128k+64:128 = u = G_masked^T w
    G bank   [128, 512]: per group k, cols 128k : 128k+128 = Gram (acc over d)
Downstream per block: one batched exp, one masked-Gram copy, four u-matmuls,
two strided muls building [w*sim | w*u], one 0/1 "E" matmul summing over m
per class -> [numer | denom], then one batched sqrt/recip/mul at the end.
"""

import os
import sys

sys.path.insert(0, "/opt/trn_rl_repo")

import ml_dtypes
import numpy as np

B, C, M, D = 64, 1000, 11, 1024
BETA = 5.5
N_CORES = 8
C_PER = C // N_CORES          # 125 classes per core
CPG = 11                      # classes per group
NG = 12                       # groups per core (132 class slots >= 125)
PG = CPG * M                  # 121 used partitions per group
DCH = D // 128                # 8 d-chunks
ROWS = NG * 128               # 1536 class-major rows per core
GPB = 4                       # groups per PSUM block
NB = NG // GPB                # 3 blocks

_cache = {}


def _build():
    import concourse.mybir as mybir
    import concourse.tile as tile
    from concourse import bacc

    f32 = mybir.dt.float32
    bf16 = mybir.dt.bfloat16

    nc = bacc.Bacc("TRN2", target_bir_lowering=False, debug=False,
                   num_devices=N_CORES)

    # membf rows: [64 img | 128 mask/em | 1536 class-major memory rows].
    # The xbar transpose of the leading rows lands imgT, the block-diagonal
    # Gram mask, and the 0/1 class-sum matrix (all bf16-exact) in exactly
    # the layouts the compute wants — no separate const loads at all.
    EXT = 192
    membf = nc.dram_tensor("membf", [EXT + ROWS, D], bf16, kind="ExternalInput")
    out = nc.dram_tensor("out", [16, NG * 64], f32, kind="ExternalOutput")

    with tile.TileContext(nc) as tc:
        with (
            tc.tile_pool(name="const", bufs=1) as const,
            tc.tile_pool(name="sb", bufs=2) as sb,
            tc.tile_pool(name="ps_su", bufs=2, space="PSUM") as ps_su,
            tc.tile_pool(name="ps_g", bufs=2, space="PSUM") as ps_g,
            tc.tile_pool(name="ps_nd", bufs=1, space="PSUM") as ps_nd,
        ):
            # memT[d % 128, d_chunk, cm]; one tile per transpose batch of 2
            # groups.  Tile 0 carries EXT extra leading cm-columns: 0:64
            # imgT, then transposed mask/em rows (mask d-chunks 0-3, em 4).
            NT = NG // 2
            mt = [const.tile([128, DCH, (EXT if q == 0 else 0) + 256],
                             bf16, name=f"mt{q}", tag=f"mt{q}")
                  for q in range(NT)]
            it = mt[0][:, :, 0:64]
            mask_bf = mt[0][:, 0:4, 64:192]          # [128, 4, 128] bf16
            em_bf = mt[0][:, 4, 64:80]               # [128, 16] bf16
            lg = const.tile([16, NG * 64], f32)
            bias_exp = const.tile([128, 1], f32)
            bias_eps = const.tile([16, 1], f32)
            nc.vector.memset(bias_exp[:], -BETA)
            nc.vector.memset(bias_eps[:], 1e-30)

            # xbar transposes first so compute starts ASAP; mask/em (needed
            # only mid-pipeline) follow.  Keeping all plain copies after all
            # transposes avoids repeated xbar-mode serialization stalls.
            for q in range(NT):
                r0 = 0 if q == 0 else EXT + q * 256
                nc.sync.dma_start(
                    mt[q][:],
                    membf.ap()[r0:EXT + (q + 1) * 256, :],
                    transpose=True,
                )
            # [numer | denom] for all 12 groups: one 3-bank PSUM tile; each
            # E-matmul writes exactly one bank-aligned [16, 512] slice
            nd_all = ps_nd.tile([16, NG * 128], f32)

            for nb in range(NB):
                su = ps_su.tile([128, GPB * 128], f32)
                gp = ps_g.tile([128, GPB * 128], f32)
                for k in range(GPB):
                    g = nb * GPB + k
                    q, r = divmod(g, 2)
                    ext = EXT if q == 0 else 0
                    for i in range(DCH):
                        blk = mt[q][:, i, ext + r * 128:ext + (r + 1) * 128]
                        nc.tensor.matmul(su[:, k * 128:k * 128 + 64],
                                         blk, it[:, i, :],
                                         start=(i == 0), stop=(i == DCH - 1),
                                         skip_group_check=True)
                        nc.tensor.matmul(gp[:, k * 128:(k + 1) * 128],
                                         blk, blk,
                                         start=(i == 0), stop=(i == DCH - 1),
                                         skip_group_check=True)

                # w = exp(beta*sim - beta) for all 4 groups at once
                su4 = su[:].rearrange("p (k t b) -> p k t b", k=GPB, t=2)
                w4 = sb.tile([128, GPB * 64], bf16, tag="w4")
                nc.scalar.activation(w4[:], su4[:, :, 0, :],
                                     mybir.ActivationFunctionType.Exp,
                                     bias=bias_exp[:], scale=BETA)

                # masked Gram -> SBUF (kills cross-class + pad entries)
                gm4 = sb.tile([128, GPB * 128], bf16, tag="gm4")
                gp4 = gp[:].rearrange("p (k j) -> p k j", k=GPB)
                nc.vector.tensor_mul(gm4[:], gp4, mask_bf)

                # u_k = G_k^T @ w_k, placed next to sim_k in the same bank
                for k in range(GPB):
                    nc.tensor.matmul(su[:, k * 128 + 64:(k + 1) * 128],
                                     gm4[:, k * 128:(k + 1) * 128],
                                     w4[:, k * 64:(k + 1) * 64],
                                     start=True, stop=True,
                                     skip_group_check=True)

                # wsq = [w*sim | w*u], one fused mul with w broadcast over t
                wsq = sb.tile([128, GPB * 128], bf16, tag="wsq")
                wq4 = wsq[:].rearrange("p (k t b) -> p k t b", k=GPB, t=2)
                w4b = w4[:].rearrange("p (k u b) -> p k u b", k=GPB, u=1) \
                    .to_broadcast((128, GPB, 2, 64))
                nc.vector.tensor_mul(wq4, su4, w4b)

                # nd[c, :] = [numer | denom] per class for the whole block
                nc.tensor.matmul(
                    nd_all[:, nb * GPB * 128:(nb + 1) * GPB * 128],
                    em_bf, wsq[:], start=True, stop=True,
                    skip_group_check=True)

            # logits = numer * 100/sqrt(denom), batched over all groups,
            # reading [numer | denom] straight out of PSUM
            nd3 = nd_all[:].rearrange("p (g t b) -> p g t b", g=NG, t=2)
            s_all = sb.tile([16, NG * 64], f32, tag="s")
            nc.scalar.activation(s_all[:], nd3[:, :, 1, :],
                                 mybir.ActivationFunctionType.Sqrt,
                                 bias=bias_eps[:], scale=1e-4)
            r_all = sb.tile([16, NG * 64], f32, tag="r")
            nc.vector.reciprocal(r_all[:], s_all[:])
            nc.vector.tensor_mul(lg[:], nd3[:, :, 0, :], r_all[:])

            nc.sync.dma_start(out.ap(), lg[:])

    nc.compile()
    return nc


def _get_nc():
    if "nc" not in _cache:
        _cache["nc"] = _build()
    return _cache["nc"]


def _prep_inputs(img_features, memorized_image_feat):
    """Host-side formatting: bf16 cast, class padding, group layout."""
    bf = ml_dtypes.bfloat16
    img_b = np.ascontiguousarray(img_features.astype(bf))          # [64, 1024]
    mem_b = memorized_image_feat.astype(bf)                        # [1000,11,1024]

    m1 = np.zeros((128, 128), np.float32)
    for c in range(CPG):
        m1[c * M:(c + 1) * M, c * M:(c + 1) * M] = 1.0
    em = np.zeros((128, 16), np.float32)
    for c in range(CPG):
        em[c * M:(c + 1) * M, c] = 1.0

    # mask/em rows for the transpose stream: transposing maskem[j, 128i+p]
    # yields m1 at d-chunks 0-3 and em^T at chunk 4
    maskem = np.zeros((128, D), bf)
    for i in range(4):
        maskem[:, i * 128:(i + 1) * 128] = m1.T
    maskem[:16, 512:640] = em.T

    in_maps = []
    for k in range(N_CORES):
        sl = mem_b[k * C_PER:(k + 1) * C_PER]                      # [125,11,1024]
        pad = np.zeros((NG * CPG, M, D), bf)
        pad[:C_PER] = sl
        grp = pad.reshape(NG, PG, D)
        full = np.zeros((NG, 128, D), bf)
        full[:, :PG] = grp
        membf = np.empty((192 + ROWS, D), bf)
        membf[:64] = img_b
        membf[64:192] = maskem
        membf[192:] = full.reshape(ROWS, D)
        in_maps.append({"membf": membf})
    return in_maps


def _gather(results):
    logits = np.empty((B, C), np.float32)
    for k in range(N_CORES):
        o = results[k]["out"].reshape(16, NG, 64)[:CPG]            # [11, 12, 64]
        o = o.transpose(1, 0, 2).reshape(NG * CPG, 64)[:C_PER]     # [125, 64]
        logits[:, k * C_PER:(k + 1) * C_PER] = o.T
    return logits


def kernel(img_features, memorized_image_feat):
    from concourse.bass_utils import run_bass_kernel_spmd

    nc = _get_nc()
    in_maps = _prep_inputs(img_features, memorized_image_feat)
    res = run_bass_kernel_spmd(nc, in_maps, core_ids=list(range(N_CORES)))
    return _gather(res.results)


# revision 44
# speedup vs baseline: 1.4956x; 1.0653x over previous
"""DualMem retrieval kernel for Trainium2 (8 NeuronCores, Bass/Tile).

Math (per reference):
    sim[b,c,m]  = <img[b], mem[c,m]>
    w           = exp(-beta * (1 - sim))
    adapt[b,c]  = sum_m mem[c,m] * w[b,c,m]
    logits[b,c] = 100 * <img[b], adapt[b,c] / ||adapt[b,c]||>

Key algebraic reduction (avoids materializing adapt [B,C,D]):
    numer[b,c]  = <img[b], adapt[b,c]> = sum_m w[b,c,m] * sim[b,c,m]
    denom[b,c]  = ||adapt[b,c]||^2     = w^T G_c w,  G_c = mem_c @ mem_c^T  (11x11 Gram)
    logits      = 100 * numer / sqrt(denom)

Sharding: classes C=1000 split 125 per core across 8 cores (mem bank fully
sharded; only img replicated).

Per-core layout: groups of 11 classes x 11 memories = 121 partitions (pad to
128); 12 groups cover 132 >= 125 class slots.  The bf16 memory bank arrives
class-major and is xbar-DMA-transposed on-chip to [d, cm].  Groups are
processed in blocks of 4 sharing PSUM banks (per-element has_written makes
disjoint column ranges in one bank legal accumulation groups; the bank-level
software check is skipped):
    su bank  [128, 512]: per group k, cols 128k+0:64  = sim (acc over d)
                                      cols 128k+64:128 = u = G_masked^T w
    G bank   [128, 512]: per group k, cols 128k : 128k+128 = Gram (acc over d)
Downstream per block: one batched exp, one masked-Gram copy, four u-matmuls,
two strided muls building [w*sim | w*u], one 0/1 "E" matmul summing over m
per class -> [numer | denom], then one batched sqrt/recip/mul at the end.
"""

import os
import sys

sys.path.insert(0, "/opt/trn_rl_repo")

import ml_dtypes
import numpy as np

B, C, M, D = 64, 1000, 11, 1024
BETA = 5.5
N_CORES = 8
C_PER = C // N_CORES          # 125 classes per core
CPG = 11                      # classes per group
NG = 12                       # groups per core (132 class slots >= 125)
PG = CPG * M                  # 121 used partitions per group
DCH = D // 128                # 8 d-chunks
ROWS = NG * 128               # 1536 class-major rows per core
GPB = 4                       # groups per PSUM block
NB = NG // GPB                # 3 blocks

_cache = {}


def _build():
    import concourse.mybir as mybir
    import concourse.tile as tile
    from concourse import bacc

    f32 = mybir.dt.float32
    bf16 = mybir.dt.bfloat16

    nc = bacc.Bacc("TRN2", target_bir_lowering=False, debug=False,
                   num_devices=N_CORES)

    # membf rows: [64 img | 128 mask/em | 1536 class-major memory rows].
    # The xbar transpose of the leading rows lands imgT, the block-diagonal
    # Gram mask, and the 0/1 class-sum matrix (all bf16-exact) in exactly
    # the layouts the compute wants — no separate const loads at all.
    EXT = 192
    membf = nc.dram_tensor("membf", [EXT + ROWS, D], bf16, kind="ExternalInput")
    out = nc.dram_tensor("out", [16, NG * 64], f32, kind="ExternalOutput")

    with tile.TileContext(nc) as tc:
        with (
            tc.tile_pool(name="const", bufs=1) as const,
            tc.tile_pool(name="sb", bufs=2) as sb,
            tc.tile_pool(name="ps_su", bufs=2, space="PSUM") as ps_su,
            tc.tile_pool(name="ps_g", bufs=2, space="PSUM") as ps_g,
            tc.tile_pool(name="ps_nd", bufs=1, space="PSUM") as ps_nd,
        ):
            # memT[d % 128, d_chunk, cm]; transpose batches sized so compute
            # can start right after img+g0 land:
            #   b0: img(64)+g0(128)  b1: mask/em(128)  b2: g1  b3: g2,g3
            #   b4..b7: g4..g11 two groups each
            bat_rows = [192, 128, 128, 256, 256, 256, 256, 256]
            mt = [const.tile([128, DCH, r], bf16, name=f"mt{q}", tag=f"mt{q}")
                  for q, r in enumerate(bat_rows)]
            # group g -> (batch tile, col offset)
            gloc = {0: (mt[0], 64), 1: (mt[2], 0), 2: (mt[3], 0),
                    3: (mt[3], 128)}
            for g in range(4, NG):
                gloc[g] = (mt[4 + (g - 4) // 2], 128 * ((g - 4) % 2))
            it = mt[0][:, :, 0:64]
            mask_bf = mt[1][:, 0:4, :]               # [128, 4, 128] bf16
            em_bf = mt[1][:, 4, 0:16]                # [128, 16] bf16
            lg = const.tile([16, NG * 64], f32)
            bias_exp = const.tile([128, 1], f32)
            bias_eps = const.tile([16, 1], f32)
            nc.vector.memset(bias_exp[:], -BETA)
            nc.vector.memset(bias_eps[:], 1e-30)

            # xbar transposes in issue order; everything (img, mask/em, mem
            # bank) rides the transpose stream — no plain input DMAs at all.
            r0 = 0
            for q, r in enumerate(bat_rows):
                nc.sync.dma_start(
                    mt[q][:],
                    membf.ap()[r0:r0 + r, :],
                    transpose=True,
                )
                r0 += r

            # [numer | denom]: blocks 0-1 share a 2-bank PSUM tile so their
            # finals run while block 2 computes; block 2 gets its own bank
            nd_a = ps_nd.tile([16, 2 * GPB * 128], f32, name="nd_a")
            nd_b = ps_nd.tile([16, GPB * 128], f32, name="nd_b")

            for nb in range(NB):
                su = ps_su.tile([128, GPB * 128], f32)
                gp = ps_g.tile([128, GPB * 128], f32)
                for k in range(GPB):
                    tile_, off = gloc[nb * GPB + k]
                    for i in range(DCH):
                        blk = tile_[:, i, off:off + 128]
                        nc.tensor.matmul(su[:, k * 128:k * 128 + 64],
                                         blk, it[:, i, :],
                                         start=(i == 0), stop=(i == DCH - 1),
                                         skip_group_check=True)
                        nc.tensor.matmul(gp[:, k * 128:(k + 1) * 128],
                                         blk, blk,
                                         start=(i == 0), stop=(i == DCH - 1),
                                         skip_group_check=True)

                # w = exp(beta*sim - beta) for all 4 groups at once
                su4 = su[:].rearrange("p (k t b) -> p k t b", k=GPB, t=2)
                w4 = sb.tile([128, GPB * 64], bf16, tag="w4")
                nc.scalar.activation(w4[:], su4[:, :, 0, :],
                                     mybir.ActivationFunctionType.Exp,
                                     bias=bias_exp[:], scale=BETA)

                # masked Gram -> SBUF (kills cross-class + pad entries)
                gm4 = sb.tile([128, GPB * 128], bf16, tag="gm4")
                gp4 = gp[:].rearrange("p (k j) -> p k j", k=GPB)
                nc.vector.tensor_mul(gm4[:], gp4, mask_bf)

                # u_k = G_k^T @ w_k, placed next to sim_k in the same bank
                for k in range(GPB):
                    nc.tensor.matmul(su[:, k * 128 + 64:(k + 1) * 128],
                                     gm4[:, k * 128:(k + 1) * 128],
                                     w4[:, k * 64:(k + 1) * 64],
                                     start=True, stop=True,
                                     skip_group_check=True)

                # wsq = [w*sim | w*u], one fused mul with w broadcast over t
                wsq = sb.tile([128, GPB * 128], bf16, tag="wsq")
                wq4 = wsq[:].rearrange("p (k t b) -> p k t b", k=GPB, t=2)
                w4b = w4[:].rearrange("p (k u b) -> p k u b", k=GPB, u=1) \
                    .to_broadcast((128, GPB, 2, 64))
                nc.vector.tensor_mul(wq4, su4, w4b)

                # nd[c, :] = [numer | denom] per class for the whole block
                dst = (nd_a[:, nb * GPB * 128:(nb + 1) * GPB * 128]
                       if nb < 2 else nd_b[:])
                nc.tensor.matmul(dst, em_bf, wsq[:], start=True, stop=True,
                                 skip_group_check=True)

            # logits = numer * 100/sqrt(denom), straight out of PSUM;
            # blocks 0-1 finalize while block 2 still computes
            for half, (nd_t, n) in enumerate([(nd_a, 2 * GPB), (nd_b, GPB)]):
                nd3 = nd_t[:].rearrange("p (g t b) -> p g t b", g=n, t=2)
                s_h = sb.tile([16, n * 64], f32, tag=f"s{half}")
                nc.scalar.activation(s_h[:], nd3[:, :, 1, :],
                                     mybir.ActivationFunctionType.Sqrt,
                                     bias=bias_eps[:], scale=1e-4)
                r_h = sb.tile([16, n * 64], f32, tag=f"r{half}")
                nc.vector.reciprocal(r_h[:], s_h[:])
                o0 = half * 2 * GPB * 64
                nc.vector.tensor_mul(lg[:, o0:o0 + n * 64], nd3[:, :, 0, :],
                                     r_h[:])
                nc.sync.dma_start(out.ap()[:, o0:o0 + n * 64],
                                 lg[:, o0:o0 + n * 64])

    nc.compile()
    return nc


def _get_nc():
    if "nc" not in _cache:
        _cache["nc"] = _build()
    return _cache["nc"]


def _prep_inputs(img_features, memorized_image_feat):
    """Host-side formatting: bf16 cast, class padding, group layout."""
    bf = ml_dtypes.bfloat16
    img_b = np.ascontiguousarray(img_features.astype(bf))          # [64, 1024]
    mem_b = memorized_image_feat.astype(bf)                        # [1000,11,1024]

    m1 = np.zeros((128, 128), np.float32)
    for c in range(CPG):
        m1[c * M:(c + 1) * M, c * M:(c + 1) * M] = 1.0
    em = np.zeros((128, 16), np.float32)
    for c in range(CPG):
        em[c * M:(c + 1) * M, c] = 1.0

    # mask/em rows for the transpose stream: transposing maskem[j, 128i+p]
    # yields m1 at d-chunks 0-3 and em^T at chunk 4
    maskem = np.zeros((128, D), bf)
    for i in range(4):
        maskem[:, i * 128:(i + 1) * 128] = m1.T
    maskem[:16, 512:640] = em.T

    in_maps = []
    for k in range(N_CORES):
        sl = mem_b[k * C_PER:(k + 1) * C_PER]                      # [125,11,1024]
        pad = np.zeros((NG * CPG, M, D), bf)
        pad[:C_PER] = sl
        grp = pad.reshape(NG, PG, D)
        full = np.zeros((NG, 128, D), bf)
        full[:, :PG] = grp
        rows = full.reshape(ROWS, D)
        membf = np.empty((192 + ROWS, D), bf)
        membf[:64] = img_b              # batch 0: img + g0
        membf[64:192] = rows[:128]
        membf[192:320] = maskem         # batch 1: mask/em
        membf[320:] = rows[128:]        # batches 2..: g1..g11
        in_maps.append({"membf": membf})
    return in_maps


def _gather(results):
    logits = np.empty((B, C), np.float32)
    for k in range(N_CORES):
        o = results[k]["out"].reshape(16, NG, 64)[:CPG]            # [11, 12, 64]
        o = o.transpose(1, 0, 2).reshape(NG * CPG, 64)[:C_PER]     # [125, 64]
        logits[:, k * C_PER:(k + 1) * C_PER] = o.T
    return logits


def kernel(img_features, memorized_image_feat):
    from concourse.bass_utils import run_bass_kernel_spmd

    nc = _get_nc()
    in_maps = _prep_inputs(img_features, memorized_image_feat)
    res = run_bass_kernel_spmd(nc, in_maps, core_ids=list(range(N_CORES)))
    return _gather(res.results)


# revision 48
# speedup vs baseline: 1.5043x; 1.0058x over previous
"""DualMem retrieval kernel for Trainium2 (8 NeuronCores, Bass/Tile).

Math (per reference):
    sim[b,c,m]  = <img[b], mem[c,m]>
    w           = exp(-beta * (1 - sim))
    adapt[b,c]  = sum_m mem[c,m] * w[b,c,m]
    logits[b,c] = 100 * <img[b], adapt[b,c] / ||adapt[b,c]||>

Key algebraic reduction (avoids materializing adapt [B,C,D]):
    numer[b,c]  = <img[b], adapt[b,c]> = sum_m w[b,c,m] * sim[b,c,m]
    denom[b,c]  = ||adapt[b,c]||^2     = w^T G_c w,  G_c = mem_c @ mem_c^T  (11x11 Gram)
    logits      = 100 * numer / sqrt(denom)

Sharding: classes C=1000 split 125 per core across 8 cores (mem bank fully
sharded; only img replicated).

Per-core layout: groups of 11 classes x 11 memories = 121 partitions (pad to
128); 12 groups cover 132 >= 125 class slots.  The bf16 memory bank arrives
class-major and is xbar-DMA-transposed on-chip to [d, cm].  Groups are
processed in blocks of 4 sharing PSUM banks (per-element has_written makes
disjoint column ranges in one bank legal accumulation groups; the bank-level
software check is skipped):
    su bank  [128, 512]: per group k, cols 128k+0:64  = sim (acc over d)
                                      cols 128k+64:128 = u = G_masked^T w
    G bank   [128, 512]: per group k, cols 128k : 128k+128 = Gram (acc over d)
Downstream per block: one batched exp, one masked-Gram copy, four u-matmuls,
two strided muls building [w*sim | w*u], one 0/1 "E" matmul summing over m
per class -> [numer | denom], then one batched sqrt/recip/mul at the end.
"""

import os
import sys

sys.path.insert(0, "/opt/trn_rl_repo")

import ml_dtypes
import numpy as np

B, C, M, D = 64, 1000, 11, 1024
BETA = 5.5
N_CORES = 8
C_PER = C // N_CORES          # 125 classes per core
CPG = 11                      # classes per group
NG = 12                       # groups per core (132 class slots >= 125)
PG = CPG * M                  # 121 used partitions per group
DCH = D // 128                # 8 d-chunks
ROWS = NG * 128               # 1536 class-major rows per core
GPB = 4                       # groups per PSUM block
NB = NG // GPB                # 3 blocks

_cache = {}


def _build():
    import concourse.mybir as mybir
    import concourse.tile as tile
    from concourse import bacc

    f32 = mybir.dt.float32
    bf16 = mybir.dt.bfloat16

    nc = bacc.Bacc("TRN2", target_bir_lowering=False, debug=False,
                   num_devices=N_CORES)

    # membf rows: [64 img | 128 mask/em | 1536 class-major memory rows].
    # The xbar transpose of the leading rows lands imgT, the block-diagonal
    # Gram mask, and the 0/1 class-sum matrix (all bf16-exact) in exactly
    # the layouts the compute wants — no separate const loads at all.
    EXT = 192
    membf = nc.dram_tensor("membf", [EXT + ROWS, D], bf16, kind="ExternalInput")
    out = nc.dram_tensor("out", [16, NG * 64], f32, kind="ExternalOutput")

    with tile.TileContext(nc) as tc:
        with (
            tc.tile_pool(name="const", bufs=1) as const,
            tc.tile_pool(name="sb", bufs=2) as sb,
            tc.tile_pool(name="ps_su", bufs=2, space="PSUM") as ps_su,
            tc.tile_pool(name="ps_g", bufs=2, space="PSUM") as ps_g,
            tc.tile_pool(name="ps_nd", bufs=1, space="PSUM") as ps_nd,
        ):
            # memT[d % 128, d_chunk, cm]; transpose batches sized so compute
            # can start right after img+g0 land:
            #   b0: img(64)+g0(128)  b1: mask/em(128)  b2: g1  b3: g2,g3
            #   b4..b7: g4..g11 two groups each
            bat_rows = [192, 128, 128, 256, 256, 256, 256, 256]
            mt = [const.tile([128, DCH, r], bf16, name=f"mt{q}", tag=f"mt{q}")
                  for q, r in enumerate(bat_rows)]
            # group g -> (batch tile, col offset)
            gloc = {0: (mt[0], 64), 1: (mt[2], 0), 2: (mt[3], 0),
                    3: (mt[3], 128)}
            for g in range(4, NG):
                gloc[g] = (mt[4 + (g - 4) // 2], 128 * ((g - 4) % 2))
            it = mt[0][:, :, 0:64]
            mask_bf = mt[1][:, 0:4, :]               # [128, 4, 128] bf16
            em_bf = mt[1][:, 4, 0:16]                # [128, 16] bf16
            lg = const.tile([16, NG * 64], f32)
            bias_exp = const.tile([128, 1], f32)
            bias_eps = const.tile([16, 1], f32)
            nc.vector.memset(bias_exp[:], -BETA)
            nc.vector.memset(bias_eps[:], 1e-30)

            # xbar transposes in issue order; everything (img, mask/em, mem
            # bank) rides the transpose stream — no plain input DMAs at all.
            r0 = 0
            for q, r in enumerate(bat_rows):
                nc.sync.dma_start(
                    mt[q][:],
                    membf.ap()[r0:r0 + r, :],
                    transpose=True,
                )
                r0 += r

            # [numer | denom]: blocks 0-1 share a 2-bank PSUM tile so their
            # finals run while block 2 computes; block 2 gets its own bank
            nd_a = ps_nd.tile([16, 2 * GPB * 128], f32, name="nd_a")
            nd_b = ps_nd.tile([16, GPB * 128], f32, name="nd_b")

            exps = []
            for nb in range(NB):
                su = ps_su.tile([128, GPB * 128], f32)
                gp = ps_g.tile([128, GPB * 128], f32)
                for k in range(GPB):
                    tile_, off = gloc[nb * GPB + k]
                    for i in range(DCH):
                        blk = tile_[:, i, off:off + 128]
                        nc.tensor.matmul(su[:, k * 128:k * 128 + 64],
                                         blk, it[:, i, :],
                                         start=(i == 0), stop=(i == DCH - 1),
                                         skip_group_check=True)
                        nc.tensor.matmul(gp[:, k * 128:(k + 1) * 128],
                                         blk, blk,
                                         start=(i == 0), stop=(i == DCH - 1),
                                         skip_group_check=True)

                # w = exp(beta*sim - beta) for all 4 groups at once
                su4 = su[:].rearrange("p (k t b) -> p k t b", k=GPB, t=2)
                w4 = sb.tile([128, GPB * 64], bf16, tag="w4")
                exps.append(nc.scalar.activation(
                    w4[:], su4[:, :, 0, :],
                    mybir.ActivationFunctionType.Exp,
                    bias=bias_exp[:], scale=BETA))

                # masked Gram -> SBUF (kills cross-class + pad entries)
                gm4 = sb.tile([128, GPB * 128], bf16, tag="gm4")
                gp4 = gp[:].rearrange("p (k j) -> p k j", k=GPB)
                nc.vector.tensor_mul(gm4[:], gp4, mask_bf)

                # u_k = G_k^T @ w_k, placed next to sim_k in the same bank
                for k in range(GPB):
                    nc.tensor.matmul(su[:, k * 128 + 64:(k + 1) * 128],
                                     gm4[:, k * 128:(k + 1) * 128],
                                     w4[:, k * 64:(k + 1) * 64],
                                     start=True, stop=True,
                                     skip_group_check=True)

                # wsq = [w*sim | w*u], one fused mul with w broadcast over t
                wsq = sb.tile([128, GPB * 128], bf16, tag="wsq")
                wq4 = wsq[:].rearrange("p (k t b) -> p k t b", k=GPB, t=2)
                w4b = w4[:].rearrange("p (k u b) -> p k u b", k=GPB, u=1) \
                    .to_broadcast((128, GPB, 2, 64))
                nc.vector.tensor_mul(wq4, su4, w4b)

                # nd[c, :] = [numer | denom] per class for the whole block
                dst = (nd_a[:, nb * GPB * 128:(nb + 1) * GPB * 128]
                       if nb < 2 else nd_b[:])
                nc.tensor.matmul(dst, em_bf, wsq[:], start=True, stop=True,
                                 skip_group_check=True)

            # logits = numer * 100/sqrt(denom), straight out of PSUM;
            # blocks 0-1 finalize while block 2 still computes
            from concourse.tile_rust import add_dep_helper
            for half, (nd_t, n) in enumerate([(nd_a, 2 * GPB), (nd_b, GPB)]):
                nd3 = nd_t[:].rearrange("p (g t b) -> p g t b", g=n, t=2)
                s_h = sb.tile([16, n * 64], f32, tag=f"s{half}")
                sq = nc.scalar.activation(s_h[:], nd3[:, :, 1, :],
                                          mybir.ActivationFunctionType.Sqrt,
                                          bias=bias_eps[:], scale=1e-4)
                # keep every Sqrt after the last Exp: one ACT table swap
                add_dep_helper(sq.ins, exps[-1].ins, sync=False,
                               reason="ACT func-table order")
                r_h = sb.tile([16, n * 64], f32, tag=f"r{half}")
                nc.vector.reciprocal(r_h[:], s_h[:])
                o0 = half * 2 * GPB * 64
                nc.vector.tensor_mul(lg[:, o0:o0 + n * 64], nd3[:, :, 0, :],
                                     r_h[:])
                nc.sync.dma_start(out.ap()[:, o0:o0 + n * 64],
                                 lg[:, o0:o0 + n * 64])

    nc.compile()
    return nc


def _get_nc():
    if "nc" not in _cache:
        _cache["nc"] = _build()
    return _cache["nc"]


def _prep_inputs(img_features, memorized_image_feat):
    """Host-side formatting: bf16 cast, class padding, group layout."""
    bf = ml_dtypes.bfloat16
    img_b = np.ascontiguousarray(img_features.astype(bf))          # [64, 1024]
    mem_b = memorized_image_feat.astype(bf)                        # [1000,11,1024]

    m1 = np.zeros((128, 128), np.float32)
    for c in range(CPG):
        m1[c * M:(c + 1) * M, c * M:(c + 1) * M] = 1.0
    em = np.zeros((128, 16), np.float32)
    for c in range(CPG):
        em[c * M:(c + 1) * M, c] = 1.0

    # mask/em rows for the transpose stream: transposing maskem[j, 128i+p]
    # yields m1 at d-chunks 0-3 and em^T at chunk 4
    maskem = np.zeros((128, D), bf)
    for i in range(4):
        maskem[:, i * 128:(i + 1) * 128] = m1.T
    maskem[:16, 512:640] = em.T

    in_maps = []
    for k in range(N_CORES):
        sl = mem_b[k * C_PER:(k + 1) * C_PER]                      # [125,11,1024]
        pad = np.zeros((NG * CPG, M, D), bf)
        pad[:C_PER] = sl
        grp = pad.reshape(NG, PG, D)
        full = np.zeros((NG, 128, D), bf)
        full[:, :PG] = grp
        rows = full.reshape(ROWS, D)
        membf = np.empty((192 + ROWS, D), bf)
        membf[:64] = img_b              # batch 0: img + g0
        membf[64:192] = rows[:128]
        membf[192:320] = maskem         # batch 1: mask/em
        membf[320:] = rows[128:]        # batches 2..: g1..g11
        in_maps.append({"membf": membf})
    return in_maps


def _gather(results):
    logits = np.empty((B, C), np.float32)
    for k in range(N_CORES):
        o = results[k]["out"].reshape(16, NG, 64)[:CPG]            # [11, 12, 64]
        o = o.transpose(1, 0, 2).reshape(NG * CPG, 64)[:C_PER]     # [125, 64]
        logits[:, k * C_PER:(k + 1) * C_PER] = o.T
    return logits


def kernel(img_features, memorized_image_feat):
    from concourse.bass_utils import run_bass_kernel_spmd

    nc = _get_nc()
    in_maps = _prep_inputs(img_features, memorized_image_feat)
    res = run_bass_kernel_spmd(nc, in_maps, core_ids=list(range(N_CORES)))
    return _gather(res.results)


# revision 56
# speedup vs baseline: 1.5687x; 1.0429x over previous
"""DualMem retrieval kernel for Trainium2 (8 NeuronCores, Bass/Tile).

Math (per reference):
    sim[b,c,m]  = <img[b], mem[c,m]>
    w           = exp(-beta * (1 - sim))
    adapt[b,c]  = sum_m mem[c,m] * w[b,c,m]
    logits[b,c] = 100 * <img[b], adapt[b,c] / ||adapt[b,c]||>

Key algebraic reduction (avoids materializing adapt [B,C,D]):
    numer[b,c]  = <img[b], adapt[b,c]> = sum_m w[b,c,m] * sim[b,c,m]
    denom[b,c]  = ||adapt[b,c]||^2     = w^T G_c w,  G_c = mem_c @ mem_c^T  (11x11 Gram)
    logits      = 100 * numer / sqrt(denom)

Sharding: classes C=1000 split 125 per core across 8 cores (mem bank fully
sharded; only img replicated).

Per-core layout: groups of 11 classes x 11 memories = 121 partitions (pad to
128); 12 groups cover 132 >= 125 class slots.  The bf16 memory bank arrives
class-major and is xbar-DMA-transposed on-chip to [d, cm]; img, the Gram
mask, and the 0/1 class-sum matrix ride the same transpose stream (all
bf16-exact), so there are no plain input DMAs at all.  Groups are processed
in blocks of [4,4,2,2] sharing PSUM banks (per-element has_written makes
disjoint column ranges in one bank legal accumulation groups; the bank-level
software check is skipped):
    su bank [128, gn*128]: per group k, cols 128k+0:64  = sim (acc over d)
                                        cols 128k+64:128 = u = G_masked^T w
    G bank  [128, gn*128]: per group k, cols 128k:128k+128 = Gram (acc over d)
Downstream per block: one batched exp, one masked-Gram copy (the mandatory
PSUM->SBUF move), gn u-matmuls, one broadcast mul building [w*sim | w*u],
one 0/1 "E" matmul summing over m per class -> [numer | denom] in PSUM.
Finals read PSUM directly and use 100/sqrt(d) = exp(-0.5*ln(d) + ln(100));
Ln and Exp share one ACT function table, so the table is loaded exactly
once.  The small trailing blocks keep the end-of-kernel dependency chain
short, and per-block finals + split output DMAs overlap earlier compute.
"""

import sys

sys.path.insert(0, "/opt/trn_rl_repo")

import ml_dtypes
import numpy as np

B, C, M, D = 64, 1000, 11, 1024
BETA = 5.5
N_CORES = 8
C_PER = C // N_CORES          # 125 classes per core
CPG = 11                      # classes per group
NG = 12                       # groups per core (132 class slots >= 125)
PG = CPG * M                  # 121 used partitions per group
DCH = D // 128                # 8 d-chunks
ROWS = NG * 128               # 1536 class-major rows per core

_cache = {}


def _build():
    import concourse.mybir as mybir
    import concourse.tile as tile
    from concourse import bacc

    # Pin every activation to the one ACT table that holds BOTH Exp and Ln
    # (indices must be preserved — empty the other sets instead of dropping
    # them) so the function table is loaded once and never swapped.
    if not getattr(bacc, "_act_tables_pinned", False):
        real = bacc.get_activation_tables

        def pinned(arch):
            return {k: (v if k == "natural_log_exp_and_others" else set())
                    for k, v in real(arch).items()}
        bacc.get_activation_tables = pinned
        bacc._act_tables_pinned = True

    f32 = mybir.dt.float32
    bf16 = mybir.dt.bfloat16

    nc = bacc.Bacc("TRN2", target_bir_lowering=False, debug=False,
                   num_devices=N_CORES)

    # membf rows: [64 img | 128 mask/em | 1536 class-major memory rows].
    # The xbar transpose of the leading rows lands imgT, the block-diagonal
    # Gram mask, and the 0/1 class-sum matrix (all bf16-exact) in exactly
    # the layouts the compute wants — no separate const loads at all.
    EXT = 192
    membf = nc.dram_tensor("membf", [EXT + ROWS, D], bf16, kind="ExternalInput")
    out = nc.dram_tensor("out", [16, NG * 64], f32, kind="ExternalOutput")

    with tile.TileContext(nc) as tc:
        with (
            tc.tile_pool(name="const", bufs=1) as const,
            tc.tile_pool(name="sb", bufs=3) as sb,
            tc.tile_pool(name="ps_su", bufs=2, space="PSUM") as ps_su,
            tc.tile_pool(name="ps_g", bufs=2, space="PSUM") as ps_g,
            tc.tile_pool(name="ps_nd", bufs=1, space="PSUM") as ps_nd,
        ):
            # memT[d % 128, d_chunk, cm]; transpose batches sized so compute
            # can start right after img+g0 land:
            #   b0: img(64)+g0(128)  b1: mask/em(128)  b2: g1  b3: g2,g3
            #   b4..b7: g4..g11 two groups each
            bat_rows = [192, 128, 128, 256, 256, 256, 256, 256]
            mt = [const.tile([128, DCH, r], bf16, name=f"mt{q}", tag=f"mt{q}")
                  for q, r in enumerate(bat_rows)]
            # group g -> (batch tile, col offset)
            gloc = {0: (mt[0], 64), 1: (mt[2], 0), 2: (mt[3], 0),
                    3: (mt[3], 128)}
            for g in range(4, NG):
                gloc[g] = (mt[4 + (g - 4) // 2], 128 * ((g - 4) % 2))
            it = mt[0][:, :, 0:64]
            mask_bf = mt[1][:, 0:4, :]               # [128, 4, 128] bf16
            em_bf = mt[1][:, 4, 0:16]                # [128, 16] bf16
            lg = const.tile([16, NG * 64], f32)
            bias_exp = const.tile([128, 1], f32)
            bias_eps = const.tile([16, 1], f32)
            bias_ln100 = const.tile([16, 1], f32)
            nc.vector.memset(bias_exp[:], -BETA)
            nc.vector.memset(bias_eps[:], 1e-30)
            nc.vector.memset(bias_ln100[:], float(np.log(100.0)))

            # xbar transposes in issue order; everything (img, mask/em, mem
            # bank) rides the transpose stream — no plain input DMAs at all.
            r0 = 0
            for q, r in enumerate(bat_rows):
                nc.sync.dma_start(
                    mt[q][:],
                    membf.ap()[r0:r0 + r, :],
                    transpose=True,
                )
                r0 += r

            # [numer | denom]: blocks 0-1 share a 2-bank PSUM tile so their
            # finals run while block 2 computes; block 2 gets its own bank
            nd_a = ps_nd.tile([16, 2 * GPB * 128], f32, name="nd_a")
            nd_b = ps_nd.tile([16, GPB * 128], f32, name="nd_b")

            exps = []
            for nb in range(NB):
                su = ps_su.tile([128, GPB * 128], f32)
                gp = ps_g.tile([128, GPB * 128], f32)
                for k in range(GPB):
                    tile_, off = gloc[nb * GPB + k]
                    for i in range(DCH):
                        blk = tile_[:, i, off:off + 128]
                        nc.tensor.matmul(su[:, k * 128:k * 128 + 64],
                                         blk, it[:, i, :],
                                         start=(i == 0), stop=(i == DCH - 1),
                                         skip_group_check=True)
                        nc.tensor.matmul(gp[:, k * 128:(k + 1) * 128],
                                         blk, blk,
                                         start=(i == 0), stop=(i == DCH - 1),
                                         skip_group_check=True)

                # w = exp(beta*sim - beta) for all 4 groups at once
                su4 = su[:].rearrange("p (k t b) -> p k t b", k=GPB, t=2)
                w4 = sb.tile([128, GPB * 64], bf16, tag="w4")
                exps.append(nc.scalar.activation(
                    w4[:], su4[:, :, 0, :],
                    mybir.ActivationFunctionType.Exp,
                    bias=bias_exp[:], scale=BETA))

                # masked Gram -> SBUF (kills cross-class + pad entries)
                gm4 = sb.tile([128, GPB * 128], bf16, tag="gm4")
                gp4 = gp[:].rearrange("p (k j) -> p k j", k=GPB)
                nc.vector.tensor_mul(gm4[:], gp4, mask_bf)

                # u_k = G_k^T @ w_k, placed next to sim_k in the same bank
                for k in range(GPB):
                    nc.tensor.matmul(su[:, k * 128 + 64:(k + 1) * 128],
                                     gm4[:, k * 128:(k + 1) * 128],
                                     w4[:, k * 64:(k + 1) * 64],
                                     start=True, stop=True,
                                     skip_group_check=True)

                # wsq = [w*sim | w*u], one fused mul with w broadcast over t
                wsq = sb.tile([128, GPB * 128], bf16, tag="wsq")
                wq4 = wsq[:].rearrange("p (k t b) -> p k t b", k=GPB, t=2)
                w4b = w4[:].rearrange("p (k u b) -> p k u b", k=GPB, u=1) \
                    .to_broadcast((128, GPB, 2, 64))
                nc.vector.tensor_mul(wq4, su4, w4b)

                # nd[c, :] = [numer | denom] per class for the whole block
                dst = (nd_a[:, nb * GPB * 128:(nb + 1) * GPB * 128]
                       if nb < 2 else nd_b[:])
                nc.tensor.matmul(dst, em_bf, wsq[:], start=True, stop=True,
                                 skip_group_check=True)

            # logits = numer * 100/sqrt(denom), straight out of PSUM;
            # blocks 0-1 finalize while block 2 still computes
            # 100/sqrt(denom) = exp(-0.5*ln(denom) + ln(100)) — Ln and Exp
            # live in the same ACT function table, so no table swap ever
            for half, (nd_t, n) in enumerate([(nd_a, 2 * GPB), (nd_b, GPB)]):
                nd3 = nd_t[:].rearrange("p (g t b) -> p g t b", g=n, t=2)
                s_h = sb.tile([16, n * 64], f32, tag=f"s{half}")
                nc.scalar.activation(s_h[:], nd3[:, :, 1, :],
                                     mybir.ActivationFunctionType.Ln,
                                     bias=bias_eps[:], scale=1.0)
                r_h = sb.tile([16, n * 64], f32, tag=f"r{half}")
                nc.scalar.activation(r_h[:], s_h[:],
                                     mybir.ActivationFunctionType.Exp,
                                     bias=bias_ln100[:], scale=-0.5)
                o0 = half * 2 * GPB * 64
                nc.vector.tensor_mul(lg[:, o0:o0 + n * 64], nd3[:, :, 0, :],
                                     r_h[:])
                nc.sync.dma_start(out.ap()[:, o0:o0 + n * 64],
                                 lg[:, o0:o0 + n * 64])

    nc.compile()
    return nc


def _get_nc():
    if "nc" not in _cache:
        _cache["nc"] = _build()
    return _cache["nc"]


def _prep_inputs(img_features, memorized_image_feat):
    """Host-side formatting: bf16 cast, class padding, group layout."""
    bf = ml_dtypes.bfloat16
    img_b = np.ascontiguousarray(img_features.astype(bf))          # [64, 1024]
    mem_b = memorized_image_feat.astype(bf)                        # [1000,11,1024]

    m1 = np.zeros((128, 128), np.float32)
    for c in range(CPG):
        m1[c * M:(c + 1) * M, c * M:(c + 1) * M] = 1.0
    em = np.zeros((128, 16), np.float32)
    for c in range(CPG):
        em[c * M:(c + 1) * M, c] = 1.0

    # mask/em rows for the transpose stream: transposing maskem[j, 128i+p]
    # yields m1 at d-chunks 0-3 and em^T at chunk 4
    maskem = np.zeros((128, D), bf)
    for i in range(4):
        maskem[:, i * 128:(i + 1) * 128] = m1.T
    maskem[:16, 512:640] = em.T

    in_maps = []
    for k in range(N_CORES):
        sl = mem_b[k * C_PER:(k + 1) * C_PER]                      # [125,11,1024]
        pad = np.zeros((NG * CPG, M, D), bf)
        pad[:C_PER] = sl
        grp = pad.reshape(NG, PG, D)
        full = np.zeros((NG, 128, D), bf)
        full[:, :PG] = grp
        rows = full.reshape(ROWS, D)
        membf = np.empty((192 + ROWS, D), bf)
        membf[:64] = img_b              # batch 0: img + g0
        membf[64:192] = rows[:128]
        membf[192:320] = maskem         # batch 1: mask/em
        membf[320:] = rows[128:]        # batches 2..: g1..g11
        in_maps.append({"membf": membf})
    return in_maps


def _gather(results):
    logits = np.empty((B, C), np.float32)
    for k in range(N_CORES):
        o = results[k]["out"].reshape(16, NG, 64)[:CPG]            # [11, 12, 64]
        o = o.transpose(1, 0, 2).reshape(NG * CPG, 64)[:C_PER]     # [125, 64]
        logits[:, k * C_PER:(k + 1) * C_PER] = o.T
    return logits


def kernel(img_features, memorized_image_feat):
    from concourse.bass_utils import run_bass_kernel_spmd

    nc = _get_nc()
    in_maps = _prep_inputs(img_features, memorized_image_feat)
    res = run_bass_kernel_spmd(nc, in_maps, core_ids=list(range(N_CORES)))
    return _gather(res.results)
